# revision 7
# baseline (speedup 1.0000x reference)
"""MaxPool3d (kernel=2, stride=2) on Trainium2, 8-core data-parallel.

Input  x: (2, 32, 128, 128, 128) f32  -> flattened to 64 channels, 8 per core.
Output y: (2, 32, 64, 64, 64) f32.

Per-core layout: one tile covers half a channel (64 input D-planes).
SBUF partition p = (d' in 0..31, hb in 0..3) where d' = output depth index
within the tile and hb = quarter-of-H block; p = 4*d' + hb is affine over
DRAM (partition stride = 4096 elements = 16 KiB contiguous run per
partition), so every DMA spans all 128 partitions with large contiguous
descriptors.

Pooling = three cascaded elementwise-max stages on VectorE, all within the
free dimension:
  1. D-pairs:  max(A0, A1) where A0/A1 are the even/odd plane loads
  2. H-pairs:  max over row pairs (step-2 row slices)
  3. W-pairs:  max over element pairs (step-2 element slices)
"""

import numpy as np

import concourse.bass as bass
import concourse.tile as tile
from concourse import bacc, mybir
from concourse import bass_utils

CPC = 8            # channels per core (64 total B*C over 8 cores)
D = H = W = 128
DT = mybir.dt.float32

_CACHE = {}


def _build_module():
    nc = bacc.Bacc("TRN2", target_bir_lowering=False, debug=False, num_devices=8)
    x = nc.dram_tensor("x", [CPC, D, H, W], DT, kind="ExternalInput").ap()
    y = nc.dram_tensor("y", [CPC, D // 2, H // 2, W // 2], DT, kind="ExternalOutput").ap()

    with tile.TileContext(nc) as tc:
        with tc.tile_pool(name="loads", bufs=3) as loads, \
             tc.tile_pool(name="work", bufs=3) as work:
            # Tile schedule: 64 input planes per tile, except the last
            # channel which is split into quarters (16 planes) so the final
            # load->compute->store tail after the last DMA is short.
            tiles = []
            for c in range(CPC - 1):
                tiles.append((c, 0, 64))
                tiles.append((c, 64, 64))
            for q in range(8):
                tiles.append((CPC - 1, q * 16, 16))

            for c, base, nplanes in tiles:
                np_out = nplanes // 2  # output planes in this tile
                nrow = np_out  # input rows per partition (128 partitions total)
                # [128p = (d', hb), nrow rows, 128 w] per plane-parity
                a0 = loads.tile([128, nrow, 128], DT, name="a0", tag="a0")
                nc.sync.dma_start(a0, x[c, base : base + nplanes : 2])
                a1 = loads.tile([128, nrow, 128], DT, name="a1", tag="a1")
                nc.sync.dma_start(a1, x[c, base + 1 : base + nplanes : 2])

                # D-pair max in place into a0 (elementwise, same AP -> safe)
                nc.vector.tensor_max(a0, a0, a1)

                hmax = work.tile([128, nrow // 2, 128], DT, name="hmax", tag="hmax")
                nc.vector.tensor_max(hmax, a0[:, 0::2, :], a0[:, 1::2, :])

                wpair = hmax.rearrange("p r (w2 t) -> p r w2 t", t=2)
                wmax = work.tile([128, nrow // 2, 64], DT, name="wmax", tag="wmax")
                nc.vector.tensor_max(wmax, wpair[:, :, :, 0], wpair[:, :, :, 1])

                # store: partition (d', hb) -> y[c, base/2+d', rows, :]
                # (plain row-major slice has the same flat element order)
                nc.scalar.dma_start(y[c, base // 2 : base // 2 + np_out], wmax)

    nc.compile()
    return nc


def _get_module():
    if "nc" not in _CACHE:
        _CACHE["nc"] = _build_module()
    return _CACHE["nc"]


def kernel(x: np.ndarray) -> np.ndarray:
    B, C, d, h, w = x.shape
    assert (B, C, d, h, w) == (2, 32, 128, 128, 128), x.shape
    nc = _get_module()

    xf = np.ascontiguousarray(x, dtype=np.float32).reshape(B * C, d, h, w)
    in_maps = [
        {"x": np.ascontiguousarray(xf[i * CPC : (i + 1) * CPC])} for i in range(8)
    ]
    res = bass_utils.run_bass_kernel_spmd(nc, in_maps, core_ids=list(range(8)))
    out = np.concatenate([r["y"] for r in res.results], axis=0)
    return out.reshape(B, C, d // 2, h // 2, w // 2)


# revision 10
# speedup vs baseline: 1.0867x; 1.0867x over previous
"""MaxPool3d (kernel=2, stride=2) on Trainium2, 8-core data-parallel.

Input  x: (2, 32, 128, 128, 128) f32  -> flattened to 64 channels, 8 per core.
Output y: (2, 32, 64, 64, 64) f32.

Per-core layout: one tile covers half a channel (64 input D-planes).
SBUF partition p = (d' in 0..31, hb in 0..3) where d' = output depth index
within the tile and hb = quarter-of-H block; p = 4*d' + hb is affine over
DRAM (partition stride = 4096 elements = 16 KiB contiguous run per
partition), so every DMA spans all 128 partitions with large contiguous
descriptors.

Pooling = three cascaded elementwise-max stages on VectorE, all within the
free dimension:
  1. D-pairs:  max(A0, A1) where A0/A1 are the even/odd plane loads
  2. H-pairs:  max over row pairs (step-2 row slices)
  3. W-pairs:  max over element pairs (step-2 element slices)
"""

import numpy as np

import concourse.bass as bass
import concourse.tile as tile
from concourse import bacc, mybir
from concourse import bass_utils

CPC = 8            # channels per core (64 total B*C over 8 cores)
D = H = W = 128
DT = mybir.dt.float32

_CACHE = {}


def _build_module():
    nc = bacc.Bacc("TRN2", target_bir_lowering=False, debug=False, num_devices=8)
    x = nc.dram_tensor("x", [CPC, D, H, W], DT, kind="ExternalInput").ap()
    y = nc.dram_tensor("y", [CPC, D // 2, H // 2, W // 2], DT, kind="ExternalOutput").ap()

    with tile.TileContext(nc) as tc:
        with tc.tile_pool(name="loads", bufs=3) as loads, \
             tc.tile_pool(name="work", bufs=3) as work:
            tiles = []
            for c in range(CPC):
                tiles.append((c, 0, 64))
                tiles.append((c, 64, 64))

            for c, base, nplanes in tiles:
                np_out = nplanes // 2  # output planes in this tile
                nrow = np_out  # input rows per partition (128 partitions total)
                # [128p = (d', hb), nrow rows, 128 w] per plane-parity
                a0 = loads.tile([128, nrow, 128], DT, name="a0", tag="a0")
                nc.sync.dma_start(a0, x[c, base : base + nplanes : 2])
                a1 = loads.tile([128, nrow, 128], DT, name="a1", tag="a1")
                nc.sync.dma_start(a1, x[c, base + 1 : base + nplanes : 2])

                # D-pair max in place into a0 (elementwise, same AP -> safe)
                nc.vector.tensor_max(a0, a0, a1)

                hmax = work.tile([128, nrow // 2, 128], DT, name="hmax", tag="hmax")
                nc.vector.tensor_max(hmax, a0[:, 0::2, :], a0[:, 1::2, :])

                wpair = hmax.rearrange("p r (w2 t) -> p r w2 t", t=2)
                wmax = work.tile([128, nrow // 2, 64], DT, name="wmax", tag="wmax")
                nc.vector.tensor_max(wmax, wpair[:, :, :, 0], wpair[:, :, :, 1])

                # store: partition (d', hb) -> y[c, base/2+d', rows, :]
                # (plain row-major slice has the same flat element order)
                nc.scalar.dma_start(y[c, base // 2 : base // 2 + np_out], wmax)

    nc.compile()
    return nc


def _get_module():
    if "nc" not in _CACHE:
        _CACHE["nc"] = _build_module()
    return _CACHE["nc"]


def kernel(x: np.ndarray) -> np.ndarray:
    B, C, d, h, w = x.shape
    assert (B, C, d, h, w) == (2, 32, 128, 128, 128), x.shape
    nc = _get_module()

    xf = np.ascontiguousarray(x, dtype=np.float32).reshape(B * C, d, h, w)
    in_maps = [
        {"x": np.ascontiguousarray(xf[i * CPC : (i + 1) * CPC])} for i in range(8)
    ]
    res = bass_utils.run_bass_kernel_spmd(nc, in_maps, core_ids=list(range(8)))
    out = np.concatenate([r["y"] for r in res.results], axis=0)
    return out.reshape(B, C, d // 2, h // 2, w // 2)
